# revision 6
# baseline (speedup 1.0000x reference)
"""Trainium2 Bass kernel for nn_AttentionBlock (GroupNorm + 1-head self-attention).

Reference computation (per batch b, C=256 channels, N=4096 spatial):
    xn = GroupNorm(x; 32 groups, eps=1e-6) * gn_w + gn_b
    q/k/v = W @ xn + b          (1x1 conv == channel matmul)
    attn  = softmax(q^T k / 16, axis=j)
    out   = x + Wo @ (v @ attn^T) + bo

Sharding: 8 cores = 4 batches x 2 query-halves. Each core computes
GroupNorm + K/V for its whole batch (duplicated across the pair) and
attention + output projection for its 2048 query rows.

Per-core x is sent with its own query columns rotated to the front
(attention is permutation-equivariant in the key/value axis j), so the
SPMD program always works on columns [0, 2048).
"""

import sys

sys.path.insert(0, "/opt/trn_rl_repo")

import numpy as np

B, C, N = 4, 256, 4096
HALF = N // 2
P = 128
NCORES = 8
GROUPS = 32
GSIZE = C // GROUPS  # 8
EPS = 1e-6
SCALE = C ** (-0.5)  # 1/16
ITILE = 512  # query-tile width
NIT = HALF // ITILE  # 4 query tiles per core
NJC = N // P  # 32 key chunks

_PROG = None
_LAST_RESULTS = None
_TRACE = False


def _build():
    import concourse.bass as bass
    import concourse.tile as tile
    from concourse import bacc, mybir

    F32 = mybir.dt.float32
    F32R = mybir.dt.float32r
    AF = mybir.ActivationFunctionType
    OP = mybir.AluOpType

    nc = bacc.Bacc("TRN2", target_bir_lowering=False, debug=False,
                   num_devices=NCORES)

    x_d = nc.declare_dram_parameter("x", [C, N], F32R, isOutput=False)
    xres_d = nc.declare_dram_parameter("xres", [C, HALF], F32, isOutput=False)
    wq_d = nc.declare_dram_parameter("wqT", [C, C], F32R, isOutput=False)
    wk_d = nc.declare_dram_parameter("wkT", [C, C], F32R, isOutput=False)
    wv_d = nc.declare_dram_parameter("wvT", [C, C], F32R, isOutput=False)
    wo_d = nc.declare_dram_parameter("woT", [C, C], F32R, isOutput=False)
    bq_d = nc.declare_dram_parameter("bq", [C], F32, isOutput=False)
    bk_d = nc.declare_dram_parameter("bk", [C], F32, isOutput=False)
    bv_d = nc.declare_dram_parameter("bv", [C], F32, isOutput=False)
    bo_d = nc.declare_dram_parameter("bo", [C], F32, isOutput=False)
    gnw_d = nc.declare_dram_parameter("gnw", [C], F32, isOutput=False)
    gnb_d = nc.declare_dram_parameter("gnb", [C], F32, isOutput=False)
    a8_d = nc.declare_dram_parameter("a8", [C, GROUPS], F32, isOutput=False)
    e8_d = nc.declare_dram_parameter("e8", [P, C], F32, isOutput=False)
    out_d = nc.declare_dram_parameter("out", [C, HALF], F32, isOutput=True)

    with tile.TileContext(nc) as tc:
        with (
            tc.tile_pool(name="big", bufs=1) as big,
            tc.tile_pool(name="small", bufs=1) as small,
            tc.tile_pool(name="pp", bufs=3) as pp,
            tc.tile_pool(name="accp", bufs=2) as accp,
            tc.tile_pool(name="op", bufs=2) as op_pool,
            tc.tile_pool(name="resp", bufs=3) as resp,
            tc.tile_pool(name="rp", bufs=2) as rp,
            tc.tile_pool(name="psS", bufs=3, space="PSUM") as psS,
            tc.tile_pool(name="psO", bufs=2, space="PSUM") as psO,
        ):
            # ---- load inputs ----
            x_sb = big.tile([P, 2, N], F32R, tag="x")
            nc.sync.dma_start(out=x_sb, in_=x_d[:].rearrange("(o p) j -> p o j", p=P))

            wq_sb = small.tile([P, 2, C], F32R, tag="wq")
            wk_sb = small.tile([P, 2, C], F32R, tag="wk")
            wv_sb = small.tile([P, 2, C], F32R, tag="wv")
            wo_sb = small.tile([P, 2, C], F32R, tag="wo")
            for t, d in [(wq_sb, wq_d), (wk_sb, wk_d), (wv_sb, wv_d), (wo_sb, wo_d)]:
                nc.sync.dma_start(out=t, in_=d[:].rearrange("(o p) c -> p o c", p=P))

            bq_sb = small.tile([P, 2], F32, tag="bq")
            bk_sb = small.tile([P, 2], F32, tag="bk")
            bo_sb = small.tile([P, 2], F32, tag="bo")
            gnw_sb = small.tile([P, 2], F32, tag="gnw")
            gnb_sb = small.tile([P, 2], F32, tag="gnb")
            for t, d in [(bq_sb, bq_d), (bk_sb, bk_d), (bo_sb, bo_d),
                         (gnw_sb, gnw_d), (gnb_sb, gnb_d)]:
                nc.sync.dma_start(out=t, in_=d[:].rearrange("(o p) -> p o", p=P))
            bv_bc = small.tile([P, C], F32, tag="bv")
            bv_ap = bv_d[:]
            nc.gpsimd.dma_start(
                out=bv_bc,
                in_=bass.AP(tensor=bv_ap.tensor, offset=bv_ap.offset,
                            ap=[[0, P], [1, C]]))

            a8_sb = small.tile([P, 2, GROUPS], F32, tag="a8")
            nc.sync.dma_start(out=a8_sb, in_=a8_d[:].rearrange("(o p) g -> p o g", p=P))
            e8_sb = small.tile([P, 2, P], F32, tag="e8")
            nc.sync.dma_start(out=e8_sb, in_=e8_d[:].rearrange("g (o m) -> g o m", m=P))

            xres_sb = big.tile([P, 2, HALF], F32, tag="xres")
            nc.sync.dma_start(out=xres_sb, in_=xres_d[:].rearrange("(o p) i -> p o i", p=P))

            ones_mat = small.tile([P, P], F32, tag="ones")
            nc.vector.memset(ones_mat, 1.0)
            eps_t = small.tile([P, 1], F32, tag="eps")
            nc.vector.memset(eps_t, EPS)

            # ---- GroupNorm stats ----
            # per-channel mean/var over the 4096 spatial positions
            rhs_stats = small.tile([P, 2, 2], F32, tag="rhs_stats")
            for o in range(2):
                stats = small.tile([P, 8, 6], F32, tag=f"bnst{o}")
                for s in range(8):
                    nc.vector.bn_stats(
                        out=stats[:, s, :],
                        in_=x_sb[:, o, s * 512:(s + 1) * 512].bitcast(F32),
                    )
                mv = small.tile([P, 2], F32, tag=f"mv{o}")
                nc.vector.bn_aggr(out=mv, in_=stats)
                # rhs_stats[:, o, 0] = mean_c ; rhs_stats[:, o, 1] = var_c + mean_c^2
                nc.vector.tensor_copy(out=rhs_stats[:, o, 0:1], in_=mv[:, 0:1])
                msq = small.tile([P, 1], F32, tag=f"msq{o}")
                nc.vector.tensor_mul(out=msq, in0=mv[:, 0:1], in1=mv[:, 0:1])
                nc.vector.tensor_add(out=rhs_stats[:, o, 1:2], in0=msq, in1=mv[:, 1:2])

            # pool 8 channels -> 32 groups:  [32, 2] = a8^T @ rhs_stats
            g_ps = psS.tile([P, ITILE], F32, tag="ps")
            nc.tensor.matmul(g_ps[0:GROUPS, 0:2], lhsT=a8_sb[:, 0], rhs=rhs_stats[:, 0],
                             start=True, stop=False)
            nc.tensor.matmul(g_ps[0:GROUPS, 0:2], lhsT=a8_sb[:, 1], rhs=rhs_stats[:, 1],
                             start=False, stop=True)
            # stats32[:, 0] = group mean, stats32[:, 1] = group rstd
            gsb = small.tile([P, 2], F32, tag="gsb")
            nc.vector.tensor_copy(out=gsb[0:GROUPS], in_=g_ps[0:GROUPS, 0:2])
            stats32 = small.tile([P, 2], F32, tag="stats32")
            nc.vector.memset(stats32, 0.0)
            nc.vector.tensor_copy(out=stats32[0:GROUPS, 0:1], in_=gsb[0:GROUPS, 0:1])
            gm2 = small.tile([P, 1], F32, tag="gm2")
            nc.vector.tensor_mul(out=gm2[0:GROUPS], in0=gsb[0:GROUPS, 0:1],
                                 in1=gsb[0:GROUPS, 0:1])
            gvar = small.tile([P, 1], F32, tag="gvar")
            nc.vector.tensor_sub(out=gvar[0:GROUPS], in0=gsb[0:GROUPS, 1:2],
                                 in1=gm2[0:GROUPS])
            gsd = small.tile([P, 1], F32, tag="gsd")
            nc.scalar.activation(out=gsd[0:GROUPS], in_=gvar[0:GROUPS], func=AF.Sqrt,
                                 bias=eps_t[0:GROUPS], scale=1.0)
            nc.vector.reciprocal(out=stats32[0:GROUPS, 1:2], in_=gsd[0:GROUPS])

            # expand 32 groups -> 256 channels, fold in gn affine:
            #   A_c = rstd_g(c) * gn_w_c ;  B_c = gn_b_c - mean_g(c) * A_c
            A_t = small.tile([P, 2], F32, tag="A")
            B_t = small.tile([P, 2], F32, tag="Bt")
            for o in range(2):
                e_ps = psS.tile([P, ITILE], F32, tag="ps")
                nc.tensor.matmul(e_ps[:, 0:2], lhsT=e8_sb[:, o], rhs=stats32,
                                 start=True, stop=True)
                nc.vector.tensor_mul(out=A_t[:, o:o + 1], in0=e_ps[:, 1:2],
                                     in1=gnw_sb[:, o:o + 1])
                mA = small.tile([P, 1], F32, tag=f"mA{o}")
                nc.vector.tensor_mul(out=mA, in0=e_ps[:, 0:1], in1=A_t[:, o:o + 1])
                nc.vector.tensor_sub(out=B_t[:, o:o + 1], in0=gnb_sb[:, o:o + 1], in1=mA)

            # apply GN in place:  xn = x * A + B
            for o in range(2):
                nc.vector.tensor_scalar(
                    out=x_sb[:, o], in0=x_sb[:, o].bitcast(F32),
                    scalar1=A_t[:, o:o + 1], scalar2=B_t[:, o:o + 1],
                    op0=OP.mult, op1=OP.add)

            # ---- projections ----
            # K: [c_out on partitions, j free], full 4096
            k_sb = big.tile([P, 2, N], F32R, tag="k")
            for oo in range(2):
                for jt in range(N // ITILE):
                    k_ps = psS.tile([P, ITILE], F32, tag="ps")
                    for ci in range(2):
                        nc.tensor.matmul(
                            k_ps, lhsT=wk_sb[:, ci, oo * P:(oo + 1) * P],
                            rhs=x_sb[:, ci, jt * ITILE:(jt + 1) * ITILE],
                            start=(ci == 0), stop=(ci == 1))
                    nc.scalar.activation(out=k_sb[:, oo, jt * ITILE:(jt + 1) * ITILE],
                                         in_=k_ps, func=AF.Identity,
                                         bias=bk_sb[:, oo:oo + 1], scale=1.0)

            # Q: only my 2048 query columns (always columns [0, 2048))
            q_sb = big.tile([P, 2, HALF], F32R, tag="q")
            for oo in range(2):
                for jt in range(NIT):
                    q_ps = psS.tile([P, ITILE], F32, tag="ps")
                    for ci in range(2):
                        nc.tensor.matmul(
                            q_ps, lhsT=wq_sb[:, ci, oo * P:(oo + 1) * P],
                            rhs=x_sb[:, ci, jt * ITILE:(jt + 1) * ITILE],
                            start=(ci == 0), stop=(ci == 1))
                    nc.scalar.activation(out=q_sb[:, oo, jt * ITILE:(jt + 1) * ITILE],
                                         in_=q_ps, func=AF.Identity,
                                         bias=bq_sb[:, oo:oo + 1], scale=1.0)

            # V^T: [j on partitions, c free], full 4096
            v_sb = big.tile([P, NJC, C], F32R, tag="v")
            for jc in range(NJC):
                v_ps = psS.tile([P, ITILE], F32, tag="ps")
                for ci in range(2):
                    nc.tensor.matmul(
                        v_ps[:, 0:C], lhsT=x_sb[:, ci, jc * P:(jc + 1) * P],
                        rhs=wv_sb[:, ci, :],
                        start=(ci == 0), stop=(ci == 1))
                nc.vector.tensor_tensor(out=v_sb[:, jc], in0=v_ps[:, 0:C],
                                        in1=bv_bc, op=OP.add)

            # ---- attention ----
            for it in range(NIT):
                isl = slice(it * ITILE, (it + 1) * ITILE)
                o_ps0 = psO.tile([P, ITILE], F32, tag="o0")
                o_ps1 = psO.tile([P, ITILE], F32, tag="o1")
                acc = accp.tile([P, ITILE], F32, tag="acc")
                for jc in range(NJC):
                    jsl = slice(jc * P, (jc + 1) * P)
                    s_ps = psS.tile([P, ITILE], F32, tag="ps")
                    nc.tensor.matmul(s_ps, lhsT=k_sb[:, 0, jsl], rhs=q_sb[:, 0, isl],
                                     start=True, stop=False)
                    nc.tensor.matmul(s_ps, lhsT=k_sb[:, 1, jsl], rhs=q_sb[:, 1, isl],
                                     start=False, stop=True)
                    p_t = pp.tile([P, ITILE], F32R, tag="p")
                    nc.scalar.activation(out=p_t, in_=s_ps, func=AF.Exp, scale=SCALE)
                    nc.tensor.matmul(o_ps0, lhsT=v_sb[:, jc, 0:P], rhs=p_t,
                                     start=(jc == 0), stop=(jc == NJC - 1))
                    nc.tensor.matmul(o_ps1, lhsT=v_sb[:, jc, P:C], rhs=p_t,
                                     start=(jc == 0), stop=(jc == NJC - 1))
                    if jc == 0:
                        nc.vector.tensor_copy(out=acc, in_=p_t.bitcast(F32))
                    else:
                        nc.vector.tensor_add(out=acc, in0=acc, in1=p_t.bitcast(F32))

                # softmax denominator, replicated across partitions by the
                # all-ones stationary operand
                l_ps = psS.tile([P, ITILE], F32, tag="ps")
                nc.tensor.matmul(l_ps, lhsT=ones_mat, rhs=acc,
                                 start=True, stop=True)
                recip = rp.tile([P, ITILE], F32, tag="recip")
                nc.vector.reciprocal(out=recip, in_=l_ps)

                o_sb = op_pool.tile([P, 2, ITILE], F32R, tag="osb")
                nc.vector.tensor_tensor(out=o_sb[:, 0], in0=o_ps0,
                                        in1=recip, op=OP.mult)
                nc.vector.tensor_tensor(out=o_sb[:, 1], in0=o_ps1,
                                        in1=recip, op=OP.mult)

                # output projection + bias + residual
                for oo in range(2):
                    u_ps = psS.tile([P, ITILE], F32, tag="ps")
                    for ci in range(2):
                        nc.tensor.matmul(
                            u_ps, lhsT=wo_sb[:, ci, oo * P:(oo + 1) * P],
                            rhs=o_sb[:, ci],
                            start=(ci == 0), stop=(ci == 1))
                    res = resp.tile([P, ITILE], F32, tag="res")
                    nc.vector.scalar_tensor_tensor(
                        out=res, in0=u_ps, scalar=bo_sb[:, oo:oo + 1],
                        in1=xres_sb[:, oo, isl], op0=OP.add, op1=OP.add)
                    nc.sync.dma_start(
                        out=out_d[:].rearrange("(o p) i -> p o i", p=P)[:, oo, isl],
                        in_=res)

    nc.compile()
    return nc


def _get_prog():
    global _PROG
    if _PROG is None:
        _PROG = _build()
    return _PROG


def kernel(x, gn_w, gn_b, wq, bq, wk, bk, wv, bv, wo, bo):
    global _LAST_RESULTS
    from concourse.bass_utils import run_bass_kernel_spmd

    nc = _get_prog()

    f32 = lambda a: np.ascontiguousarray(np.asarray(a), dtype=np.float32)
    x = f32(x).reshape(B, C, N)
    shared = {
        "wqT": f32(wq).T.copy(), "wkT": f32(wk).T.copy(),
        "wvT": f32(wv).T.copy(), "woT": f32(wo).T.copy(),
        "bq": f32(bq), "bk": f32(bk), "bv": f32(bv), "bo": f32(bo),
        "gnw": f32(gn_w), "gnb": f32(gn_b),
    }
    a8 = np.zeros((C, GROUPS), np.float32)
    a8[np.arange(C), np.arange(C) // GSIZE] = 1.0 / GSIZE
    shared["a8"] = a8
    e8 = np.zeros((P, C), np.float32)
    e8[np.arange(C) // GSIZE, np.arange(C)] = 1.0
    shared["e8"] = e8

    in_maps = []
    for core in range(NCORES):
        b, h = core // 2, core % 2
        xb = x[b]
        if h == 0:
            xc = xb
        else:
            xc = np.ascontiguousarray(np.concatenate([xb[:, HALF:], xb[:, :HALF]], axis=1))
        m = dict(shared)
        m["x"] = xc
        m["xres"] = np.ascontiguousarray(xb[:, h * HALF:(h + 1) * HALF])
        in_maps.append(m)

    _LAST_RESULTS = run_bass_kernel_spmd(nc, in_maps, list(range(NCORES)),
                                         trace=_TRACE)
    out = np.empty((B, C, N), np.float32)
    for core in range(NCORES):
        b, h = core // 2, core % 2
        out[b, :, h * HALF:(h + 1) * HALF] = _LAST_RESULTS.results[core]["out"]
    return out.reshape(B, C, 16, 16, 16)


# revision 7
# speedup vs baseline: 1.0826x; 1.0826x over previous
"""Trainium2 Bass kernel for nn_AttentionBlock (GroupNorm + 1-head self-attention).

Reference computation (per batch b, C=256 channels, N=4096 spatial):
    xn = GroupNorm(x; 32 groups, eps=1e-6) * gn_w + gn_b
    q/k/v = W @ xn + b          (1x1 conv == channel matmul)
    attn  = softmax(q^T k / 16, axis=j)
    out   = x + Wo @ (v @ attn^T) + bo

Sharding: 8 cores = 4 batches x 2 query-halves. Each core computes
GroupNorm + K/V for its whole batch (duplicated across the pair) and
attention + output projection for its 2048 query rows.

Per-core x is sent with its own query columns rotated to the front
(attention is permutation-equivariant in the key/value axis j), so the
SPMD program always works on columns [0, 2048).

Numerics: GroupNorm stats in fp32; matmul operands in bf16 (PE streams
1 column/cycle vs ~1.9 for f32r); all matmul accumulation in fp32 PSUM;
softmax sums in fp32. Scores are bounded (|s|/16 <~ 1) so exp() skips
the max-subtraction pass.

Schedule: the attention inner loop is software-pipelined - chunk j+1's
score matmuls are emitted before chunk j's PV matmuls so the in-order
PE queue never waits on the ACT exp; each query-tile's softmax/output
tail is deferred into the next tile's first chunks.
"""

import sys

sys.path.insert(0, "/opt/trn_rl_repo")

import numpy as np

B, C, N = 4, 256, 4096
HALF = N // 2
P = 128
NCORES = 8
GROUPS = 32
GSIZE = C // GROUPS  # 8
EPS = 1e-6
SCALE = C ** (-0.5)  # 1/16
ITILE = 512  # query-tile width
NIT = HALF // ITILE  # 4 query tiles per core
NJC = N // P  # 32 key chunks

_PROG = None
_LAST_RESULTS = None
_TRACE = False


def _build():
    import concourse.bass as bass
    import concourse.tile as tile
    from concourse import bacc, mybir

    F32 = mybir.dt.float32
    F32R = mybir.dt.float32r
    BF16 = mybir.dt.bfloat16
    AF = mybir.ActivationFunctionType
    OP = mybir.AluOpType

    nc = bacc.Bacc("TRN2", target_bir_lowering=False, debug=False,
                   num_devices=NCORES)

    x_d = nc.declare_dram_parameter("x", [C, N], F32, isOutput=False)
    xres_d = nc.declare_dram_parameter("xres", [C, HALF], F32, isOutput=False)
    wq_d = nc.declare_dram_parameter("wqT", [C, C], BF16, isOutput=False)
    wk_d = nc.declare_dram_parameter("wkT", [C, C], BF16, isOutput=False)
    wv_d = nc.declare_dram_parameter("wvT", [C, C], BF16, isOutput=False)
    wo_d = nc.declare_dram_parameter("woT", [C, C], BF16, isOutput=False)
    bq_d = nc.declare_dram_parameter("bq", [C], F32, isOutput=False)
    bk_d = nc.declare_dram_parameter("bk", [C], F32, isOutput=False)
    bv_d = nc.declare_dram_parameter("bv", [C], F32, isOutput=False)
    bo_d = nc.declare_dram_parameter("bo", [C], F32, isOutput=False)
    gnw_d = nc.declare_dram_parameter("gnw", [C], F32, isOutput=False)
    gnb_d = nc.declare_dram_parameter("gnb", [C], F32, isOutput=False)
    a8_d = nc.declare_dram_parameter("a8", [C, GROUPS], F32, isOutput=False)
    e8_d = nc.declare_dram_parameter("e8", [P, C], F32, isOutput=False)
    out_d = nc.declare_dram_parameter("out", [C, HALF], F32, isOutput=True)

    with tile.TileContext(nc) as tc:
        with (
            tc.tile_pool(name="big", bufs=1) as big,
            tc.tile_pool(name="small", bufs=1) as small,
            tc.tile_pool(name="pp", bufs=4) as pp,
            tc.tile_pool(name="accp", bufs=2) as accp,
            tc.tile_pool(name="op", bufs=2) as op_pool,
            tc.tile_pool(name="resp", bufs=3) as resp,
            tc.tile_pool(name="rp", bufs=2) as rp,
            tc.tile_pool(name="psS", bufs=3, space="PSUM") as psS,
            tc.tile_pool(name="psO", bufs=2, space="PSUM") as psO,
        ):
            # ---- load inputs ----
            x_sb = big.tile([P, 2, N], F32, tag="x")
            # split the 4 MB x DMA per channel-chunk so GroupNorm stats can
            # start on chunk 0 while chunk 1 is still in flight
            x_re = x_d[:].rearrange("(o p) j -> p o j", p=P)
            for o in range(2):
                nc.sync.dma_start(out=x_sb[:, o], in_=x_re[:, o])

            wq_sb = small.tile([P, 2, C], BF16, tag="wq")
            wk_sb = small.tile([P, 2, C], BF16, tag="wk")
            wv_sb = small.tile([P, 2, C], BF16, tag="wv")
            wo_sb = small.tile([P, 2, C], BF16, tag="wo")
            for t, d in [(wq_sb, wq_d), (wk_sb, wk_d), (wv_sb, wv_d), (wo_sb, wo_d)]:
                nc.sync.dma_start(out=t, in_=d[:].rearrange("(o p) c -> p o c", p=P))

            bq_sb = small.tile([P, 2], F32, tag="bq")
            bk_sb = small.tile([P, 2], F32, tag="bk")
            bo_sb = small.tile([P, 2], F32, tag="bo")
            gnw_sb = small.tile([P, 2], F32, tag="gnw")
            gnb_sb = small.tile([P, 2], F32, tag="gnb")
            for t, d in [(bq_sb, bq_d), (bk_sb, bk_d), (bo_sb, bo_d),
                         (gnw_sb, gnw_d), (gnb_sb, gnb_d)]:
                nc.sync.dma_start(out=t, in_=d[:].rearrange("(o p) -> p o", p=P))
            bv_bc = small.tile([P, C], F32, tag="bv")
            bv_ap = bv_d[:]
            nc.gpsimd.dma_start(
                out=bv_bc,
                in_=bass.AP(tensor=bv_ap.tensor, offset=bv_ap.offset,
                            ap=[[0, P], [1, C]]))

            a8_sb = small.tile([P, 2, GROUPS], F32, tag="a8")
            nc.sync.dma_start(out=a8_sb, in_=a8_d[:].rearrange("(o p) g -> p o g", p=P))
            e8_sb = small.tile([P, 2, P], F32, tag="e8")
            nc.sync.dma_start(out=e8_sb, in_=e8_d[:].rearrange("g (o m) -> g o m", m=P))

            xres_sb = big.tile([P, 2, HALF], F32, tag="xres")
            nc.sync.dma_start(out=xres_sb, in_=xres_d[:].rearrange("(o p) i -> p o i", p=P))

            ones_mat = small.tile([P, P], F32, tag="ones")
            nc.vector.memset(ones_mat, 1.0)
            eps_t = small.tile([P, 1], F32, tag="eps")
            nc.vector.memset(eps_t, EPS)

            # ---- GroupNorm stats (fp32) ----
            # per-channel mean/var over the 4096 spatial positions
            rhs_stats = small.tile([P, 2, 2], F32, tag="rhs_stats")
            for o in range(2):
                stats = small.tile([P, 8, 6], F32, tag=f"bnst{o}")
                for s in range(8):
                    nc.vector.bn_stats(
                        out=stats[:, s, :],
                        in_=x_sb[:, o, s * 512:(s + 1) * 512],
                    )
                mv = small.tile([P, 2], F32, tag=f"mv{o}")
                nc.vector.bn_aggr(out=mv, in_=stats)
                # rhs_stats[:, o, 0] = mean_c ; rhs_stats[:, o, 1] = var_c + mean_c^2
                nc.vector.tensor_copy(out=rhs_stats[:, o, 0:1], in_=mv[:, 0:1])
                msq = small.tile([P, 1], F32, tag=f"msq{o}")
                nc.vector.tensor_mul(out=msq, in0=mv[:, 0:1], in1=mv[:, 0:1])
                nc.vector.tensor_add(out=rhs_stats[:, o, 1:2], in0=msq, in1=mv[:, 1:2])

            # pool 8 channels -> 32 groups:  [32, 2] = a8^T @ rhs_stats
            g_ps = psS.tile([P, ITILE], F32, tag="ps")
            nc.tensor.matmul(g_ps[0:GROUPS, 0:2], lhsT=a8_sb[:, 0], rhs=rhs_stats[:, 0],
                             start=True, stop=False)
            nc.tensor.matmul(g_ps[0:GROUPS, 0:2], lhsT=a8_sb[:, 1], rhs=rhs_stats[:, 1],
                             start=False, stop=True)
            # stats32[:, 0] = group mean, stats32[:, 1] = group rstd
            gsb = small.tile([P, 2], F32, tag="gsb")
            nc.vector.tensor_copy(out=gsb[0:GROUPS], in_=g_ps[0:GROUPS, 0:2])
            stats32 = small.tile([P, 2], F32, tag="stats32")
            nc.vector.memset(stats32, 0.0)
            nc.vector.tensor_copy(out=stats32[0:GROUPS, 0:1], in_=gsb[0:GROUPS, 0:1])
            gm2 = small.tile([P, 1], F32, tag="gm2")
            nc.vector.tensor_mul(out=gm2[0:GROUPS], in0=gsb[0:GROUPS, 0:1],
                                 in1=gsb[0:GROUPS, 0:1])
            gvar = small.tile([P, 1], F32, tag="gvar")
            nc.vector.tensor_sub(out=gvar[0:GROUPS], in0=gsb[0:GROUPS, 1:2],
                                 in1=gm2[0:GROUPS])
            gsd = small.tile([P, 1], F32, tag="gsd")
            nc.scalar.activation(out=gsd[0:GROUPS], in_=gvar[0:GROUPS], func=AF.Sqrt,
                                 bias=eps_t[0:GROUPS], scale=1.0)
            nc.vector.reciprocal(out=stats32[0:GROUPS, 1:2], in_=gsd[0:GROUPS])

            # expand 32 groups -> 256 channels, fold in gn affine:
            #   A_c = rstd_g(c) * gn_w_c ;  B_c = gn_b_c - mean_g(c) * A_c
            A_t = small.tile([P, 2], F32, tag="A")
            B_t = small.tile([P, 2], F32, tag="Bt")
            for o in range(2):
                e_ps = psS.tile([P, ITILE], F32, tag="ps")
                nc.tensor.matmul(e_ps[:, 0:2], lhsT=e8_sb[:, o], rhs=stats32,
                                 start=True, stop=True)
                nc.vector.tensor_mul(out=A_t[:, o:o + 1], in0=e_ps[:, 1:2],
                                     in1=gnw_sb[:, o:o + 1])
                mA = small.tile([P, 1], F32, tag=f"mA{o}")
                nc.vector.tensor_mul(out=mA, in0=e_ps[:, 0:1], in1=A_t[:, o:o + 1])
                nc.vector.tensor_sub(out=B_t[:, o:o + 1], in0=gnb_sb[:, o:o + 1], in1=mA)

            # apply GN:  xn = x * A + B   (bf16, feeds all projections)
            xn_sb = big.tile([P, 2, N], BF16, tag="xn")
            for o in range(2):
                nc.vector.tensor_scalar(
                    out=xn_sb[:, o], in0=x_sb[:, o],
                    scalar1=A_t[:, o:o + 1], scalar2=B_t[:, o:o + 1],
                    op0=OP.mult, op1=OP.add)

            # ---- projections (bf16 operands, fp32 accumulate) ----
            # K: [c_out on partitions, j free], full 4096
            k_sb = big.tile([P, 2, N], BF16, tag="k")
            for oo in range(2):
                for jt in range(N // ITILE):
                    k_ps = psS.tile([P, ITILE], F32, tag="ps")
                    for ci in range(2):
                        nc.tensor.matmul(
                            k_ps, lhsT=wk_sb[:, ci, oo * P:(oo + 1) * P],
                            rhs=xn_sb[:, ci, jt * ITILE:(jt + 1) * ITILE],
                            start=(ci == 0), stop=(ci == 1))
                    nc.scalar.activation(out=k_sb[:, oo, jt * ITILE:(jt + 1) * ITILE],
                                         in_=k_ps, func=AF.Identity,
                                         bias=bk_sb[:, oo:oo + 1], scale=1.0)

            # Q: only my 2048 query columns (always columns [0, 2048))
            q_sb = big.tile([P, 2, HALF], BF16, tag="q")
            for oo in range(2):
                for jt in range(NIT):
                    q_ps = psS.tile([P, ITILE], F32, tag="ps")
                    for ci in range(2):
                        nc.tensor.matmul(
                            q_ps, lhsT=wq_sb[:, ci, oo * P:(oo + 1) * P],
                            rhs=xn_sb[:, ci, jt * ITILE:(jt + 1) * ITILE],
                            start=(ci == 0), stop=(ci == 1))
                    nc.scalar.activation(out=q_sb[:, oo, jt * ITILE:(jt + 1) * ITILE],
                                         in_=q_ps, func=AF.Identity,
                                         bias=bq_sb[:, oo:oo + 1], scale=1.0)

            # V^T: [j on partitions, c free], full 4096
            v_sb = big.tile([P, NJC, C], BF16, tag="v")
            for jc in range(NJC):
                v_ps = psS.tile([P, ITILE], F32, tag="ps")
                for ci in range(2):
                    nc.tensor.matmul(
                        v_ps[:, 0:C], lhsT=xn_sb[:, ci, jc * P:(jc + 1) * P],
                        rhs=wv_sb[:, ci, :],
                        start=(ci == 0), stop=(ci == 1))
                nc.vector.tensor_tensor(out=v_sb[:, jc], in0=v_ps[:, 0:C],
                                        in1=bv_bc, op=OP.add)

            # ---- attention (software-pipelined) ----
            def emit_s_exp(it, jc):
                """Scores S^T[j-chunk, i-tile] and P = exp(S*scale)."""
                isl = slice(it * ITILE, (it + 1) * ITILE)
                jsl = slice(jc * P, (jc + 1) * P)
                s_ps = psS.tile([P, ITILE], F32, tag="ps")
                nc.tensor.matmul(s_ps, lhsT=k_sb[:, 0, jsl], rhs=q_sb[:, 0, isl],
                                 start=True, stop=False)
                nc.tensor.matmul(s_ps, lhsT=k_sb[:, 1, jsl], rhs=q_sb[:, 1, isl],
                                 start=False, stop=True)
                p_t = pp.tile([P, ITILE], BF16, tag="p")
                nc.scalar.activation(out=p_t, in_=s_ps, func=AF.Exp, scale=SCALE)
                return p_t

            def emit_pv(jc, p_t, o_ps0, o_ps1, acc):
                nc.tensor.matmul(o_ps0, lhsT=v_sb[:, jc, 0:P], rhs=p_t,
                                 start=(jc == 0), stop=(jc == NJC - 1))
                nc.tensor.matmul(o_ps1, lhsT=v_sb[:, jc, P:C], rhs=p_t,
                                 start=(jc == 0), stop=(jc == NJC - 1))
                if jc == 0:
                    nc.vector.tensor_copy(out=acc, in_=p_t)
                else:
                    nc.vector.tensor_add(out=acc, in0=acc, in1=p_t)

            def make_tail(it, o_ps0, o_ps1, acc):
                def tail():
                    isl = slice(it * ITILE, (it + 1) * ITILE)
                    # softmax denominator, replicated across partitions by
                    # the all-ones stationary operand
                    l_ps = psS.tile([P, ITILE], F32, tag="ps")
                    nc.tensor.matmul(l_ps, lhsT=ones_mat, rhs=acc,
                                     start=True, stop=True)
                    recip = rp.tile([P, ITILE], F32, tag="recip")
                    nc.vector.reciprocal(out=recip, in_=l_ps)

                    o_sb = op_pool.tile([P, 2, ITILE], BF16, tag="osb")
                    nc.vector.tensor_tensor(out=o_sb[:, 0], in0=o_ps0,
                                            in1=recip, op=OP.mult)
                    nc.vector.tensor_tensor(out=o_sb[:, 1], in0=o_ps1,
                                            in1=recip, op=OP.mult)

                    # output projection + bias + residual
                    for oo in range(2):
                        u_ps = psS.tile([P, ITILE], F32, tag="ps")
                        for ci in range(2):
                            nc.tensor.matmul(
                                u_ps, lhsT=wo_sb[:, ci, oo * P:(oo + 1) * P],
                                rhs=o_sb[:, ci],
                                start=(ci == 0), stop=(ci == 1))
                        res = resp.tile([P, ITILE], F32, tag="res")
                        nc.vector.scalar_tensor_tensor(
                            out=res, in0=u_ps, scalar=bo_sb[:, oo:oo + 1],
                            in1=xres_sb[:, oo, isl], op0=OP.add, op1=OP.add)
                        nc.sync.dma_start(
                            out=out_d[:].rearrange("(o p) i -> p o i", p=P)[:, oo, isl],
                            in_=res)
                return tail

            pending_tail = None
            for it in range(NIT):
                o_ps0 = psO.tile([P, ITILE], F32, tag="o0")
                o_ps1 = psO.tile([P, ITILE], F32, tag="o1")
                acc = accp.tile([P, ITILE], F32, tag="acc")
                prev_p = None
                for jc in range(NJC):
                    p_t = emit_s_exp(it, jc)
                    if prev_p is not None:
                        emit_pv(jc - 1, prev_p, o_ps0, o_ps1, acc)
                    if pending_tail is not None and jc == 2:
                        pending_tail()
                        pending_tail = None
                    prev_p = p_t
                emit_pv(NJC - 1, prev_p, o_ps0, o_ps1, acc)
                pending_tail = make_tail(it, o_ps0, o_ps1, acc)
            pending_tail()

    nc.compile()
    return nc


def _get_prog():
    global _PROG
    if _PROG is None:
        _PROG = _build()
    return _PROG


def kernel(x, gn_w, gn_b, wq, bq, wk, bk, wv, bv, wo, bo):
    global _LAST_RESULTS
    import ml_dtypes
    from concourse.bass_utils import run_bass_kernel_spmd

    nc = _get_prog()

    f32 = lambda a: np.ascontiguousarray(np.asarray(a), dtype=np.float32)
    bf16 = lambda a: np.ascontiguousarray(np.asarray(a, dtype=np.float32).astype(ml_dtypes.bfloat16))
    x = f32(x).reshape(B, C, N)
    shared = {
        "wqT": bf16(np.asarray(wq).T), "wkT": bf16(np.asarray(wk).T),
        "wvT": bf16(np.asarray(wv).T), "woT": bf16(np.asarray(wo).T),
        "bq": f32(bq), "bk": f32(bk), "bv": f32(bv), "bo": f32(bo),
        "gnw": f32(gn_w), "gnb": f32(gn_b),
    }
    a8 = np.zeros((C, GROUPS), np.float32)
    a8[np.arange(C), np.arange(C) // GSIZE] = 1.0 / GSIZE
    shared["a8"] = a8
    e8 = np.zeros((P, C), np.float32)
    e8[np.arange(C) // GSIZE, np.arange(C)] = 1.0
    shared["e8"] = e8

    in_maps = []
    for core in range(NCORES):
        b, h = core // 2, core % 2
        xb = x[b]
        if h == 0:
            xc = xb
        else:
            xc = np.ascontiguousarray(np.concatenate([xb[:, HALF:], xb[:, :HALF]], axis=1))
        m = dict(shared)
        m["x"] = xc
        m["xres"] = np.ascontiguousarray(xb[:, h * HALF:(h + 1) * HALF])
        in_maps.append(m)

    _LAST_RESULTS = run_bass_kernel_spmd(nc, in_maps, list(range(NCORES)),
                                         trace=_TRACE)
    out = np.empty((B, C, N), np.float32)
    for core in range(NCORES):
        b, h = core // 2, core % 2
        out[b, :, h * HALF:(h + 1) * HALF] = _LAST_RESULTS.results[core]["out"]
    return out.reshape(B, C, 16, 16, 16)


# revision 11
# speedup vs baseline: 1.2955x; 1.1966x over previous
"""Trainium2 Bass kernel for nn_AttentionBlock (GroupNorm + 1-head self-attention).

Reference computation (per batch b, C=256 channels, N=4096 spatial):
    xn = GroupNorm(x; 32 groups, eps=1e-6) * gn_w + gn_b
    q/k/v = W @ xn + b          (1x1 conv == channel matmul)
    attn  = softmax(q^T k / 16, axis=j)
    out   = x + Wo @ (v @ attn^T) + bo

Sharding: 8 cores = 4 batches x 2 query-halves. Each core computes
GroupNorm + K/V for its whole batch (duplicated across the pair) and
attention + output projection for its 2048 query rows.

Per-core x is sent with its own query columns rotated to the front
(attention is permutation-equivariant in the key/value axis j), so the
SPMD program always works on columns [0, 2048).

Numerics: matmul operands in bf16 (PE streams 1 column/cycle), fp32
PSUM accumulation everywhere, softmax row-sums in fp32. GroupNorm is
never materialized: its affine (xn = A*x + B, A/B fp32 from bf16-x
stats) is folded into the projection weights on device:
    W @ (A*x + B) + b  ==  (W . A_col) @ x + (W @ B + b)
Scores are bounded (|s|/16 <~ 1) so exp() skips max-subtraction.

Schedule: attention is software-pipelined with a 2-chunk lookahead
(chunk j's PV matmuls are emitted after chunk j+2's score matmuls) so
the in-order PE queue never waits on the ACT exp; each query-tile's
softmax/output tail is split in two and deferred into the next tile's
early chunks.
"""

import sys

sys.path.insert(0, "/opt/trn_rl_repo")

import numpy as np

B, C, N = 4, 256, 4096
HALF = N // 2
P = 128
NCORES = 8
GROUPS = 32
GSIZE = C // GROUPS  # 8
EPS = 1e-6
SCALE = C ** (-0.5)  # 1/16
ITILE = 512  # query-tile width
NIT = HALF // ITILE  # 4 query tiles per core
NJC = N // P  # 32 key chunks

_PROG = None
_LAST_RESULTS = None
_TRACE = False


def _build():
    import concourse.bass as bass
    import concourse.tile as tile
    from concourse import bacc, mybir

    F32 = mybir.dt.float32
    F32R = mybir.dt.float32r
    BF16 = mybir.dt.bfloat16
    AF = mybir.ActivationFunctionType
    OP = mybir.AluOpType

    nc = bacc.Bacc("TRN2", target_bir_lowering=False, debug=False,
                   num_devices=NCORES)

    xbf_d = nc.declare_dram_parameter("xbf", [C, N], BF16, isOutput=False)
    xres_d = nc.declare_dram_parameter("xres", [C, HALF], F32, isOutput=False)
    wq_d = nc.declare_dram_parameter("wqT", [C, C], BF16, isOutput=False)
    wk_d = nc.declare_dram_parameter("wkT", [C, C], BF16, isOutput=False)
    wv_d = nc.declare_dram_parameter("wvT", [C, C], BF16, isOutput=False)
    wo_d = nc.declare_dram_parameter("woT", [C, C], BF16, isOutput=False)
    bq_d = nc.declare_dram_parameter("bq", [C], F32, isOutput=False)
    bk_d = nc.declare_dram_parameter("bk", [C], F32, isOutput=False)
    bv_d = nc.declare_dram_parameter("bv", [C], F32, isOutput=False)
    bo_d = nc.declare_dram_parameter("bo", [C], F32, isOutput=False)
    gnw_d = nc.declare_dram_parameter("gnw", [C], F32, isOutput=False)
    gnb_d = nc.declare_dram_parameter("gnb", [C], F32, isOutput=False)
    a8_d = nc.declare_dram_parameter("a8", [C, GROUPS], F32, isOutput=False)
    e8_d = nc.declare_dram_parameter("e8", [P, C], F32, isOutput=False)
    out_d = nc.declare_dram_parameter("out", [C, HALF], F32, isOutput=True)

    with tile.TileContext(nc) as tc:
        with (
            tc.tile_pool(name="big", bufs=1) as big,
            tc.tile_pool(name="small", bufs=1) as small,
            tc.tile_pool(name="pp", bufs=4) as pp,
            tc.tile_pool(name="accp", bufs=2) as accp,
            tc.tile_pool(name="op", bufs=2) as op_pool,
            tc.tile_pool(name="resp", bufs=3) as resp,
            tc.tile_pool(name="rp", bufs=2) as rp,
            tc.tile_pool(name="psS", bufs=3, space="PSUM") as psS,
            tc.tile_pool(name="psO", bufs=2, space="PSUM") as psO,
        ):
            # ---- load inputs ----
            # x (bf16) in 512-column slices on two DMA queues so GroupNorm
            # stats run while later slices are still in flight
            x_sb = big.tile([P, 2, N], BF16, tag="x")
            x_re = xbf_d[:].rearrange("(o p) j -> p o j", p=P)
            for s in range(8):
                for o in range(2):
                    eng = nc.sync if (s % 2 == 0) else nc.gpsimd
                    eng.dma_start(out=x_sb[:, o, s * 512:(s + 1) * 512],
                                  in_=x_re[:, o, s * 512:(s + 1) * 512])

            wq_sb = small.tile([P, 2, C], BF16, tag="wq")
            wk_sb = small.tile([P, 2, C], BF16, tag="wk")
            wv_sb = small.tile([P, 2, C], BF16, tag="wv")
            wo_sb = small.tile([P, 2, C], BF16, tag="wo")
            for t, d in [(wq_sb, wq_d), (wk_sb, wk_d), (wv_sb, wv_d), (wo_sb, wo_d)]:
                nc.sync.dma_start(out=t, in_=d[:].rearrange("(o p) c -> p o c", p=P))

            bq_sb = small.tile([P, 2], F32, tag="bq")
            bk_sb = small.tile([P, 2], F32, tag="bk")
            bo_sb = small.tile([P, 2], F32, tag="bo")
            gnw_sb = small.tile([P, 2], F32, tag="gnw")
            gnb_sb = small.tile([P, 2], F32, tag="gnb")
            for t, d in [(bq_sb, bq_d), (bk_sb, bk_d), (bo_sb, bo_d),
                         (gnw_sb, gnw_d), (gnb_sb, gnb_d)]:
                nc.gpsimd.dma_start(out=t, in_=d[:].rearrange("(o p) -> p o", p=P))
            bv_row = small.tile([1, C], F32, tag="bvr")
            nc.gpsimd.dma_start(out=bv_row, in_=bv_d[:].unsqueeze(0))

            a8_sb = small.tile([P, 2, GROUPS], F32, tag="a8")
            nc.gpsimd.dma_start(out=a8_sb, in_=a8_d[:].rearrange("(o p) g -> p o g", p=P))
            e8_sb = small.tile([P, 2, P], F32, tag="e8")
            nc.gpsimd.dma_start(out=e8_sb, in_=e8_d[:].rearrange("g (o m) -> g o m", m=P))

            ones_f32 = small.tile([P, P], F32, tag="onesf")
            nc.vector.memset(ones_f32, 1.0)
            ones_mat = small.tile([P, P], F32R, tag="ones")
            nc.vector.tensor_copy(out=ones_mat, in_=ones_f32)
            ones_row = small.tile([1, P], BF16, tag="onesr")
            nc.vector.memset(ones_row, 1.0)
            eps_t = small.tile([P, 1], F32, tag="eps")
            nc.vector.memset(eps_t, EPS)

            # ---- GroupNorm stats (fp32 accumulation over bf16 x) ----
            # per-channel mean/var over the 4096 spatial positions
            rhs_stats = small.tile([P, 2, 2], F32, tag="rhs_stats")
            for o in range(2):
                stats = small.tile([P, 8, 6], F32, tag=f"bnst{o}")
                for s in range(8):
                    nc.vector.bn_stats(
                        out=stats[:, s, :],
                        in_=x_sb[:, o, s * 512:(s + 1) * 512],
                    )
                mv = small.tile([P, 2], F32, tag=f"mv{o}")
                nc.vector.bn_aggr(out=mv, in_=stats)
                # rhs_stats[:, o, 0] = mean_c ; rhs_stats[:, o, 1] = var_c + mean_c^2
                nc.vector.tensor_copy(out=rhs_stats[:, o, 0:1], in_=mv[:, 0:1])
                msq = small.tile([P, 1], F32, tag=f"msq{o}")
                nc.vector.tensor_mul(out=msq, in0=mv[:, 0:1], in1=mv[:, 0:1])
                nc.vector.tensor_add(out=rhs_stats[:, o, 1:2], in0=msq, in1=mv[:, 1:2])

            # pool 8 channels -> 32 groups:  [32, 2] = a8^T @ rhs_stats
            g_ps = psS.tile([P, ITILE], F32, tag="ps")
            nc.tensor.matmul(g_ps[0:GROUPS, 0:2], lhsT=a8_sb[:, 0], rhs=rhs_stats[:, 0],
                             start=True, stop=False)
            nc.tensor.matmul(g_ps[0:GROUPS, 0:2], lhsT=a8_sb[:, 1], rhs=rhs_stats[:, 1],
                             start=False, stop=True)
            # stats32[:, 0] = group mean, stats32[:, 1] = group rstd
            gsb = small.tile([P, 2], F32, tag="gsb")
            nc.vector.tensor_copy(out=gsb[0:GROUPS], in_=g_ps[0:GROUPS, 0:2])
            stats32 = small.tile([P, 2], F32, tag="stats32")
            nc.vector.memset(stats32, 0.0)
            nc.vector.tensor_copy(out=stats32[0:GROUPS, 0:1], in_=gsb[0:GROUPS, 0:1])
            gm2 = small.tile([P, 1], F32, tag="gm2")
            nc.vector.tensor_mul(out=gm2[0:GROUPS], in0=gsb[0:GROUPS, 0:1],
                                 in1=gsb[0:GROUPS, 0:1])
            gvar = small.tile([P, 1], F32, tag="gvar")
            nc.vector.tensor_sub(out=gvar[0:GROUPS], in0=gsb[0:GROUPS, 1:2],
                                 in1=gm2[0:GROUPS])
            gsd = small.tile([P, 1], F32, tag="gsd")
            nc.scalar.activation(out=gsd[0:GROUPS], in_=gvar[0:GROUPS], func=AF.Sqrt,
                                 bias=eps_t[0:GROUPS], scale=1.0)
            nc.vector.reciprocal(out=stats32[0:GROUPS, 1:2], in_=gsd[0:GROUPS])

            # expand 32 groups -> 256 channels, fold in gn affine:
            #   A_c = rstd_g(c) * gn_w_c ;  B_c = gn_b_c - mean_g(c) * A_c
            A_t = small.tile([P, 2], F32, tag="A")
            B_t = small.tile([P, 2], F32, tag="Bt")
            for o in range(2):
                e_ps = psS.tile([P, ITILE], F32, tag="ps")
                nc.tensor.matmul(e_ps[:, 0:2], lhsT=e8_sb[:, o], rhs=stats32,
                                 start=True, stop=True)
                nc.vector.tensor_mul(out=A_t[:, o:o + 1], in0=e_ps[:, 1:2],
                                     in1=gnw_sb[:, o:o + 1])
                mA = small.tile([P, 1], F32, tag=f"mA{o}")
                nc.vector.tensor_mul(out=mA, in0=e_ps[:, 0:1], in1=A_t[:, o:o + 1])
                nc.vector.tensor_sub(out=B_t[:, o:o + 1], in0=gnb_sb[:, o:o + 1], in1=mA)
            B_bf = small.tile([P, 2], BF16, tag="Bbf")
            nc.vector.tensor_copy(out=B_bf, in_=B_t)

            # ---- fold GN affine into projection weights + biases ----
            # folded bias:  b' = W @ B + b   (uses the unscaled weights)
            bq2 = small.tile([P, 2], F32, tag="bq2")
            bk2 = small.tile([P, 2], F32, tag="bk2")
            for (w_sb, b_sb, b2) in [(wq_sb, bq_sb, bq2), (wk_sb, bk_sb, bk2)]:
                for oo in range(2):
                    bps = psS.tile([P, ITILE], F32, tag="ps")
                    for ci in range(2):
                        nc.tensor.matmul(bps[:, 0:1],
                                         lhsT=w_sb[:, ci, oo * P:(oo + 1) * P],
                                         rhs=B_bf[:, ci:ci + 1],
                                         start=(ci == 0), stop=(ci == 1))
                    nc.vector.tensor_add(out=b2[:, oo:oo + 1], in0=bps[:, 0:1],
                                         in1=b_sb[:, oo:oo + 1])
            # v bias in row form: bv'_row = B^T @ wvT + bv, then broadcast
            # across partitions with a K=1 ones matmul
            bvps = psS.tile([P, ITILE], F32, tag="ps")
            for ci in range(2):
                nc.tensor.matmul(bvps[0:1, 0:C], lhsT=B_bf[:, ci:ci + 1],
                                 rhs=wv_sb[:, ci, :],
                                 start=(ci == 0), stop=(ci == 1))
            bv2row = small.tile([1, C], BF16, tag="bv2r")
            nc.vector.tensor_add(out=bv2row, in0=bvps[0:1, 0:C], in1=bv_row)
            bcps = psS.tile([P, ITILE], F32, tag="ps")
            nc.tensor.matmul(bcps[:, 0:C], lhsT=ones_row, rhs=bv2row,
                             start=True, stop=True)
            bv_bc = small.tile([P, C], F32, tag="bvbc")
            nc.vector.tensor_copy(out=bv_bc, in_=bcps[:, 0:C])

            # scale weights in place:  W'T[c', o] = WT[c', o] * A[c']
            for w_sb in (wq_sb, wk_sb, wv_sb):
                for ci in range(2):
                    nc.vector.tensor_scalar_mul(out=w_sb[:, ci], in0=w_sb[:, ci],
                                                scalar1=A_t[:, ci:ci + 1])

            # ---- projections (bf16 operands, fp32 accumulate) ----
            k_sb = big.tile([P, 2, N], BF16, tag="k")
            q_sb = big.tile([P, 2, HALF], BF16, tag="q")
            for jt in range(N // ITILE):
                jts = slice(jt * ITILE, (jt + 1) * ITILE)
                for oo in range(2):
                    k_ps = psS.tile([P, ITILE], F32, tag="ps")
                    for ci in range(2):
                        nc.tensor.matmul(
                            k_ps, lhsT=wk_sb[:, ci, oo * P:(oo + 1) * P],
                            rhs=x_sb[:, ci, jts],
                            start=(ci == 0), stop=(ci == 1))
                    nc.scalar.activation(out=k_sb[:, oo, jts],
                                         in_=k_ps, func=AF.Identity,
                                         bias=bk2[:, oo:oo + 1], scale=1.0)
                if jt < NIT:
                    # Q: only my 2048 query columns (always columns [0, 2048))
                    for oo in range(2):
                        q_ps = psS.tile([P, ITILE], F32, tag="ps")
                        for ci in range(2):
                            nc.tensor.matmul(
                                q_ps, lhsT=wq_sb[:, ci, oo * P:(oo + 1) * P],
                                rhs=x_sb[:, ci, jts],
                                start=(ci == 0), stop=(ci == 1))
                        # evac on DVE to keep ACT free for the K evacuations
                        nc.vector.tensor_scalar_add(
                            out=q_sb[:, oo, jts], in0=q_ps,
                            scalar1=bq2[:, oo:oo + 1])

            # V^T: [j on partitions, c free], full 4096
            v_sb = big.tile([P, NJC, C], BF16, tag="v")
            for jc in range(NJC):
                v_ps = psS.tile([P, ITILE], F32, tag="ps")
                for ci in range(2):
                    nc.tensor.matmul(
                        v_ps[:, 0:C], lhsT=x_sb[:, ci, jc * P:(jc + 1) * P],
                        rhs=wv_sb[:, ci, :],
                        start=(ci == 0), stop=(ci == 1))
                nc.vector.tensor_tensor(out=v_sb[:, jc], in0=v_ps[:, 0:C],
                                        in1=bv_bc, op=OP.add)

            # residual input, only needed from the first attention tail on
            xres_sb = big.tile([P, 2, HALF], F32, tag="xres")
            nc.sync.dma_start(out=xres_sb, in_=xres_d[:].rearrange("(o p) i -> p o i", p=P))

            # ---- attention (software-pipelined) ----
            def emit_s_exp(it, jc):
                """Scores S^T[j-chunk, i-tile] and P = exp(S*scale)."""
                isl = slice(it * ITILE, (it + 1) * ITILE)
                jsl = slice(jc * P, (jc + 1) * P)
                s_ps = psS.tile([P, ITILE], F32, tag="ps")
                nc.tensor.matmul(s_ps, lhsT=k_sb[:, 0, jsl], rhs=q_sb[:, 0, isl],
                                 start=True, stop=False)
                nc.tensor.matmul(s_ps, lhsT=k_sb[:, 1, jsl], rhs=q_sb[:, 1, isl],
                                 start=False, stop=True)
                p_t = pp.tile([P, ITILE], BF16, tag="p")
                nc.scalar.activation(out=p_t, in_=s_ps, func=AF.Exp, scale=SCALE)
                return p_t

            def emit_pv(jc, p_t, o_ps0, o_ps1, acc):
                nc.tensor.matmul(o_ps0, lhsT=v_sb[:, jc, 0:P], rhs=p_t,
                                 start=(jc == 0), stop=(jc == NJC - 1))
                nc.tensor.matmul(o_ps1, lhsT=v_sb[:, jc, P:C], rhs=p_t,
                                 start=(jc == 0), stop=(jc == NJC - 1))
                if jc == 0:
                    nc.vector.tensor_copy(out=acc, in_=p_t)
                else:
                    nc.vector.tensor_add(out=acc, in0=acc.bitcast(F32), in1=p_t)

            def make_tail_a(it, o_ps0, o_ps1, acc):
                def tail_a():
                    # softmax denominator, replicated across partitions by
                    # the all-ones stationary operand (f32r: 1 cyc/row)
                    l_ps = psS.tile([P, ITILE], F32, tag="ps")
                    nc.tensor.matmul(l_ps, lhsT=ones_mat, rhs=acc,
                                     start=True, stop=True)
                    recip = rp.tile([P, ITILE], F32, tag="recip")
                    nc.vector.reciprocal_approx_fast(out=recip, in_=l_ps)
                    o_sb = op_pool.tile([P, 2, ITILE], BF16, tag="osb")
                    nc.vector.tensor_tensor(out=o_sb[:, 0], in0=o_ps0,
                                            in1=recip, op=OP.mult)
                    nc.vector.tensor_tensor(out=o_sb[:, 1], in0=o_ps1,
                                            in1=recip, op=OP.mult)
                    return o_sb
                return tail_a

            def make_tail_b(it, o_sb):
                def tail_b():
                    isl = slice(it * ITILE, (it + 1) * ITILE)
                    # output projection + bias + residual
                    for oo in range(2):
                        u_ps = psS.tile([P, ITILE], F32, tag="ps")
                        for ci in range(2):
                            nc.tensor.matmul(
                                u_ps, lhsT=wo_sb[:, ci, oo * P:(oo + 1) * P],
                                rhs=o_sb[:, ci],
                                start=(ci == 0), stop=(ci == 1))
                        res = resp.tile([P, ITILE], F32, tag="res")
                        nc.vector.scalar_tensor_tensor(
                            out=res, in0=u_ps, scalar=bo_sb[:, oo:oo + 1],
                            in1=xres_sb[:, oo, isl], op0=OP.add, op1=OP.add)
                        nc.sync.dma_start(
                            out=out_d[:].rearrange("(o p) i -> p o i", p=P)[:, oo, isl],
                            in_=res)
                return tail_b

            pending_a = pending_b = None
            for it in range(NIT):
                o_ps0 = psO.tile([P, ITILE], F32, tag="o0")
                o_ps1 = psO.tile([P, ITILE], F32, tag="o1")
                acc = accp.tile([P, ITILE], F32R, tag="acc")
                pq = []
                for jc in range(NJC):
                    pq.append(emit_s_exp(it, jc))
                    if jc >= 2:
                        emit_pv(jc - 2, pq[jc - 2], o_ps0, o_ps1, acc)
                    if pending_a is not None and jc == 3:
                        o_sb_prev = pending_a()
                        pending_b = make_tail_b(it - 1, o_sb_prev)
                        pending_a = None
                    if pending_b is not None and jc == 8:
                        pending_b()
                        pending_b = None
                emit_pv(NJC - 2, pq[NJC - 2], o_ps0, o_ps1, acc)
                emit_pv(NJC - 1, pq[NJC - 1], o_ps0, o_ps1, acc)
                pending_a = make_tail_a(it, o_ps0, o_ps1, acc)
            o_sb_last = pending_a()
            make_tail_b(NIT - 1, o_sb_last)()

    nc.compile()
    return nc


def _get_prog():
    global _PROG
    if _PROG is None:
        _PROG = _build()
    return _PROG


def kernel(x, gn_w, gn_b, wq, bq, wk, bk, wv, bv, wo, bo):
    global _LAST_RESULTS
    import ml_dtypes
    from concourse.bass_utils import run_bass_kernel_spmd

    nc = _get_prog()

    f32 = lambda a: np.ascontiguousarray(np.asarray(a), dtype=np.float32)
    bf16 = lambda a: np.ascontiguousarray(np.asarray(a, dtype=np.float32).astype(ml_dtypes.bfloat16))
    x = f32(x).reshape(B, C, N)
    shared = {
        "wqT": bf16(np.asarray(wq).T), "wkT": bf16(np.asarray(wk).T),
        "wvT": bf16(np.asarray(wv).T), "woT": bf16(np.asarray(wo).T),
        "bq": f32(bq), "bk": f32(bk), "bv": f32(bv), "bo": f32(bo),
        "gnw": f32(gn_w), "gnb": f32(gn_b),
    }
    a8 = np.zeros((C, GROUPS), np.float32)
    a8[np.arange(C), np.arange(C) // GSIZE] = 1.0 / GSIZE
    shared["a8"] = a8
    e8 = np.zeros((P, C), np.float32)
    e8[np.arange(C) // GSIZE, np.arange(C)] = 1.0
    shared["e8"] = e8

    in_maps = []
    for core in range(NCORES):
        b, h = core // 2, core % 2
        xb = x[b]
        if h == 0:
            xc = xb
        else:
            xc = np.ascontiguousarray(np.concatenate([xb[:, HALF:], xb[:, :HALF]], axis=1))
        m = dict(shared)
        m["xbf"] = bf16(xc)
        m["xres"] = np.ascontiguousarray(xb[:, h * HALF:(h + 1) * HALF])
        in_maps.append(m)

    _LAST_RESULTS = run_bass_kernel_spmd(nc, in_maps, list(range(NCORES)),
                                         trace=_TRACE)
    out = np.empty((B, C, N), np.float32)
    for core in range(NCORES):
        b, h = core // 2, core % 2
        out[b, :, h * HALF:(h + 1) * HALF] = _LAST_RESULTS.results[core]["out"]
    return out.reshape(B, C, 16, 16, 16)


# revision 12
# speedup vs baseline: 1.5837x; 1.2224x over previous
"""Trainium2 Bass kernel for nn_AttentionBlock (GroupNorm + 1-head self-attention).

Reference computation (per batch b, C=256 channels, N=4096 spatial):
    xn = GroupNorm(x; 32 groups, eps=1e-6) * gn_w + gn_b
    q/k/v = W @ xn + b          (1x1 conv == channel matmul)
    attn  = softmax(q^T k / 16, axis=j)
    out   = x + Wo @ (v @ attn^T) + bo

Sharding: 8 cores = 4 batches x 2 query-halves. Each core computes
GroupNorm + K/V for its whole batch (duplicated across the pair) and
attention + output projection for its 2048 query rows.

Per-core x is sent with its own query columns rotated to the front
(attention is permutation-equivariant in the key/value axis j), so the
SPMD program always works on columns [0, 2048).

Numerics: matmul operands in bf16 (PE streams 1 column/cycle), fp32
PSUM accumulation everywhere, softmax row-sums in fp32. GroupNorm is
never materialized: its affine (xn = A*x + B, A/B fp32 from bf16-x
stats) is folded into the projection weights on device:
    W @ (A*x + B) + b  ==  (W . A_col) @ x + (W @ B + b)
Scores are bounded (|s|/16 <~ 1) so exp() skips max-subtraction.

Schedule: attention is software-pipelined with a 2-chunk lookahead
(chunk j's PV matmuls are emitted after chunk j+2's score matmuls) so
the in-order PE queue never waits on the ACT exp; each query-tile's
softmax/output tail is split in two and deferred into the next tile's
early chunks.
"""

import sys

sys.path.insert(0, "/opt/trn_rl_repo")

import numpy as np

B, C, N = 4, 256, 4096
HALF = N // 2
P = 128
NCORES = 8
GROUPS = 32
GSIZE = C // GROUPS  # 8
EPS = 1e-6
SCALE = C ** (-0.5)  # 1/16
ITILE = 512  # query-tile width
NIT = HALF // ITILE  # 4 query tiles per core
NJC = N // P  # 32 key chunks

_PROG = None
_LAST_RESULTS = None
_TRACE = False


def _build():
    import concourse.bass as bass
    import concourse.tile as tile
    from concourse import bacc, mybir

    F32 = mybir.dt.float32
    F32R = mybir.dt.float32r
    BF16 = mybir.dt.bfloat16
    FP8 = mybir.dt.float8e4
    DR = mybir.MatmulPerfMode.DoubleRow
    AF = mybir.ActivationFunctionType
    OP = mybir.AluOpType

    nc = bacc.Bacc("TRN2", target_bir_lowering=False, debug=False,
                   num_devices=NCORES)

    xbf_d = nc.declare_dram_parameter("xbf", [C, N], BF16, isOutput=False)
    xres_d = nc.declare_dram_parameter("xres", [C, HALF], F32, isOutput=False)
    wq_d = nc.declare_dram_parameter("wqT", [C, C], BF16, isOutput=False)
    wk_d = nc.declare_dram_parameter("wkT", [C, C], BF16, isOutput=False)
    wv_d = nc.declare_dram_parameter("wvT", [C, C], BF16, isOutput=False)
    wo_d = nc.declare_dram_parameter("woT", [C, C], BF16, isOutput=False)
    bq_d = nc.declare_dram_parameter("bq", [C], F32, isOutput=False)
    bk_d = nc.declare_dram_parameter("bk", [C], F32, isOutput=False)
    bv_d = nc.declare_dram_parameter("bv", [C], F32, isOutput=False)
    bo_d = nc.declare_dram_parameter("bo", [C], F32, isOutput=False)
    gnw_d = nc.declare_dram_parameter("gnw", [C], F32, isOutput=False)
    gnb_d = nc.declare_dram_parameter("gnb", [C], F32, isOutput=False)
    a8_d = nc.declare_dram_parameter("a8", [C, GROUPS], F32, isOutput=False)
    e8_d = nc.declare_dram_parameter("e8", [P, C], F32, isOutput=False)
    out_d = nc.declare_dram_parameter("out", [C, HALF], F32, isOutput=True)

    with tile.TileContext(nc) as tc:
        with (
            tc.tile_pool(name="big", bufs=1) as big,
            tc.tile_pool(name="small", bufs=1) as small,
            tc.tile_pool(name="pp", bufs=4) as pp,
            tc.tile_pool(name="accp", bufs=2) as accp,
            tc.tile_pool(name="op", bufs=2) as op_pool,
            tc.tile_pool(name="resp", bufs=3) as resp,
            tc.tile_pool(name="rp", bufs=2) as rp,
            tc.tile_pool(name="psS", bufs=2, space="PSUM") as psS,
            tc.tile_pool(name="psO", bufs=1, space="PSUM") as psO,
            tc.tile_pool(name="psL", bufs=2, space="PSUM") as psL,
        ):
            # ---- load inputs ----
            # x (bf16) in 512-column slices on two DMA queues so GroupNorm
            # stats run while later slices are still in flight
            x_sb = big.tile([P, 2, N], BF16, tag="x")
            x_re = xbf_d[:].rearrange("(o p) j -> p o j", p=P)
            for s in range(8):
                for o in range(2):
                    eng = nc.sync if (s % 2 == 0) else nc.gpsimd
                    eng.dma_start(out=x_sb[:, o, s * 512:(s + 1) * 512],
                                  in_=x_re[:, o, s * 512:(s + 1) * 512])

            wq_sb = small.tile([P, 2, C], BF16, tag="wq")
            wk_sb = small.tile([P, 2, C], BF16, tag="wk")
            wv_sb = small.tile([P, 2, C], BF16, tag="wv")
            wo_sb = small.tile([P, 2, C], BF16, tag="wo")
            for t, d in [(wq_sb, wq_d), (wk_sb, wk_d), (wv_sb, wv_d), (wo_sb, wo_d)]:
                nc.sync.dma_start(out=t, in_=d[:].rearrange("(o p) c -> p o c", p=P))

            bq_sb = small.tile([P, 2], F32, tag="bq")
            bk_sb = small.tile([P, 2], F32, tag="bk")
            bo_sb = small.tile([P, 2], F32, tag="bo")
            gnw_sb = small.tile([P, 2], F32, tag="gnw")
            gnb_sb = small.tile([P, 2], F32, tag="gnb")
            for t, d in [(bq_sb, bq_d), (bk_sb, bk_d), (bo_sb, bo_d),
                         (gnw_sb, gnw_d), (gnb_sb, gnb_d)]:
                nc.gpsimd.dma_start(out=t, in_=d[:].rearrange("(o p) -> p o", p=P))
            bv_row = small.tile([1, C], F32, tag="bvr")
            nc.gpsimd.dma_start(out=bv_row, in_=bv_d[:].unsqueeze(0))

            a8_sb = small.tile([P, 2, GROUPS], F32, tag="a8")
            nc.gpsimd.dma_start(out=a8_sb, in_=a8_d[:].rearrange("(o p) g -> p o g", p=P))
            e8_sb = small.tile([P, 2, P], F32, tag="e8")
            nc.gpsimd.dma_start(out=e8_sb, in_=e8_d[:].rearrange("g (o m) -> g o m", m=P))

            ones_f32 = small.tile([P, 2, P], F32, tag="onesf")
            nc.vector.memset(ones_f32, 1.0)
            ones8 = small.tile([P, 2, P], FP8, tag="ones8")
            nc.vector.tensor_copy(out=ones8, in_=ones_f32)
            ones_row = small.tile([1, P], BF16, tag="onesr")
            nc.vector.memset(ones_row, 1.0)
            eps_t = small.tile([P, 1], F32, tag="eps")
            nc.vector.memset(eps_t, EPS)

            def ps1():
                t = psS.tile([P, 2, ITILE], F32, tag="ps", name="ps1b")
                return t[:, 0, :]

            # ---- GroupNorm stats (fp32 accumulation over bf16 x) ----
            # per-channel mean/var over the 4096 spatial positions
            rhs_stats = small.tile([P, 2, 2], F32, tag="rhs_stats")
            for o in range(2):
                stats = small.tile([P, 8, 6], F32, tag=f"bnst{o}")
                for s in range(8):
                    nc.vector.bn_stats(
                        out=stats[:, s, :],
                        in_=x_sb[:, o, s * 512:(s + 1) * 512],
                    )
                mv = small.tile([P, 2], F32, tag=f"mv{o}")
                nc.vector.bn_aggr(out=mv, in_=stats)
                # rhs_stats[:, o, 0] = mean_c ; rhs_stats[:, o, 1] = var_c + mean_c^2
                nc.vector.tensor_copy(out=rhs_stats[:, o, 0:1], in_=mv[:, 0:1])
                msq = small.tile([P, 1], F32, tag=f"msq{o}")
                nc.vector.tensor_mul(out=msq, in0=mv[:, 0:1], in1=mv[:, 0:1])
                nc.vector.tensor_add(out=rhs_stats[:, o, 1:2], in0=msq, in1=mv[:, 1:2])

            # pool 8 channels -> 32 groups:  [32, 2] = a8^T @ rhs_stats
            g_ps = ps1()
            nc.tensor.matmul(g_ps[0:GROUPS, 0:2], lhsT=a8_sb[:, 0], rhs=rhs_stats[:, 0],
                             start=True, stop=False)
            nc.tensor.matmul(g_ps[0:GROUPS, 0:2], lhsT=a8_sb[:, 1], rhs=rhs_stats[:, 1],
                             start=False, stop=True)
            # stats32[:, 0] = group mean, stats32[:, 1] = group rstd
            gsb = small.tile([P, 2], F32, tag="gsb")
            nc.vector.tensor_copy(out=gsb[0:GROUPS], in_=g_ps[0:GROUPS, 0:2])
            stats32 = small.tile([P, 2], F32, tag="stats32")
            nc.vector.memset(stats32, 0.0)
            nc.vector.tensor_copy(out=stats32[0:GROUPS, 0:1], in_=gsb[0:GROUPS, 0:1])
            gm2 = small.tile([P, 1], F32, tag="gm2")
            nc.vector.tensor_mul(out=gm2[0:GROUPS], in0=gsb[0:GROUPS, 0:1],
                                 in1=gsb[0:GROUPS, 0:1])
            gvar = small.tile([P, 1], F32, tag="gvar")
            nc.vector.tensor_sub(out=gvar[0:GROUPS], in0=gsb[0:GROUPS, 1:2],
                                 in1=gm2[0:GROUPS])
            gsd = small.tile([P, 1], F32, tag="gsd")
            nc.scalar.activation(out=gsd[0:GROUPS], in_=gvar[0:GROUPS], func=AF.Sqrt,
                                 bias=eps_t[0:GROUPS], scale=1.0)
            nc.vector.reciprocal(out=stats32[0:GROUPS, 1:2], in_=gsd[0:GROUPS])

            # expand 32 groups -> 256 channels, fold in gn affine:
            #   A_c = rstd_g(c) * gn_w_c ;  B_c = gn_b_c - mean_g(c) * A_c
            A_t = small.tile([P, 2], F32, tag="A")
            B_t = small.tile([P, 2], F32, tag="Bt")
            for o in range(2):
                e_ps = ps1()
                nc.tensor.matmul(e_ps[:, 0:2], lhsT=e8_sb[:, o], rhs=stats32,
                                 start=True, stop=True)
                nc.vector.tensor_mul(out=A_t[:, o:o + 1], in0=e_ps[:, 1:2],
                                     in1=gnw_sb[:, o:o + 1])
                mA = small.tile([P, 1], F32, tag=f"mA{o}")
                nc.vector.tensor_mul(out=mA, in0=e_ps[:, 0:1], in1=A_t[:, o:o + 1])
                nc.vector.tensor_sub(out=B_t[:, o:o + 1], in0=gnb_sb[:, o:o + 1], in1=mA)
            B_bf = small.tile([P, 2], BF16, tag="Bbf")
            nc.vector.tensor_copy(out=B_bf, in_=B_t)

            # ---- fold GN affine into projection weights + biases ----
            # folded bias:  b' = W @ B + b   (uses the unscaled weights)
            bq2 = small.tile([P, 2], F32, tag="bq2")
            bk2 = small.tile([P, 2], F32, tag="bk2")
            for (w_sb, b_sb, b2) in [(wq_sb, bq_sb, bq2), (wk_sb, bk_sb, bk2)]:
                for oo in range(2):
                    bps = ps1()
                    for ci in range(2):
                        nc.tensor.matmul(bps[:, 0:1],
                                         lhsT=w_sb[:, ci, oo * P:(oo + 1) * P],
                                         rhs=B_bf[:, ci:ci + 1],
                                         start=(ci == 0), stop=(ci == 1))
                    nc.vector.tensor_add(out=b2[:, oo:oo + 1], in0=bps[:, 0:1],
                                         in1=b_sb[:, oo:oo + 1])
            # v bias in row form: bv'_row = B^T @ wvT + bv, then broadcast
            # across partitions with a K=1 ones matmul
            bvps = ps1()
            for ci in range(2):
                nc.tensor.matmul(bvps[0:1, 0:C], lhsT=B_bf[:, ci:ci + 1],
                                 rhs=wv_sb[:, ci, :],
                                 start=(ci == 0), stop=(ci == 1))
            bv2row = small.tile([1, C], BF16, tag="bv2r")
            nc.vector.tensor_add(out=bv2row, in0=bvps[0:1, 0:C], in1=bv_row)
            bcps = ps1()
            nc.tensor.matmul(bcps[:, 0:C], lhsT=ones_row, rhs=bv2row,
                             start=True, stop=True)
            bv_bc = small.tile([P, C], F32, tag="bvbc")
            nc.vector.tensor_copy(out=bv_bc, in_=bcps[:, 0:C])

            # scale weights in place:  W'T[c', o] = WT[c', o] * A[c']
            for w_sb in (wq_sb, wk_sb, wv_sb):
                for ci in range(2):
                    nc.vector.tensor_scalar_mul(out=w_sb[:, ci], in0=w_sb[:, ci],
                                                scalar1=A_t[:, ci:ci + 1])

            # ---- projections (bf16 operands, fp32 accumulate) ----
            k_sb = big.tile([P, 2, N], FP8, tag="k")
            q_sb = big.tile([P, 2, HALF], FP8, tag="q")
            for jt in range(N // ITILE):
                jts = slice(jt * ITILE, (jt + 1) * ITILE)
                for oo in range(2):
                    k_ps = ps1()
                    for ci in range(2):
                        nc.tensor.matmul(
                            k_ps, lhsT=wk_sb[:, ci, oo * P:(oo + 1) * P],
                            rhs=x_sb[:, ci, jts],
                            start=(ci == 0), stop=(ci == 1))
                    nc.scalar.activation(out=k_sb[:, oo, jts],
                                         in_=k_ps, func=AF.Identity,
                                         bias=bk2[:, oo:oo + 1], scale=1.0)
                if jt < NIT:
                    # Q: only my 2048 query columns (always columns [0, 2048))
                    for oo in range(2):
                        q_ps = ps1()
                        for ci in range(2):
                            nc.tensor.matmul(
                                q_ps, lhsT=wq_sb[:, ci, oo * P:(oo + 1) * P],
                                rhs=x_sb[:, ci, jts],
                                start=(ci == 0), stop=(ci == 1))
                        # evac on DVE to keep ACT free for the K evacuations
                        nc.vector.tensor_scalar_add(
                            out=q_sb[:, oo, jts], in0=q_ps,
                            scalar1=bq2[:, oo:oo + 1])

            # V^T: [j on partitions, c free], full 4096
            v_sb = big.tile([P, NJC, C], FP8, tag="v")
            for jc in range(NJC):
                v_ps = ps1()
                for ci in range(2):
                    nc.tensor.matmul(
                        v_ps[:, 0:C], lhsT=x_sb[:, ci, jc * P:(jc + 1) * P],
                        rhs=wv_sb[:, ci, :],
                        start=(ci == 0), stop=(ci == 1))
                nc.vector.tensor_tensor(out=v_sb[:, jc], in0=v_ps[:, 0:C],
                                        in1=bv_bc, op=OP.add)

            # residual input, only needed from the first attention tail on
            xres_sb = big.tile([P, 2, HALF], F32, tag="xres")
            nc.sync.dma_start(out=xres_sb, in_=xres_d[:].rearrange("(o p) i -> p o i", p=P))

            # ---- attention (fp8 DoubleRow, software-pipelined in chunk pairs) ----
            NPR = NJC // 2  # 16 key-chunk pairs

            def emit_s_exp(it, pr):
                """Scores for chunk pair (2pr, 2pr+1) and P = exp(S*scale).

                Each DoubleRow matmul contracts the full C=256 via the fp8
                k-interleave; one ACT exp covers both chunks (2 PSUM banks).
                """
                isl = slice(it * ITILE, (it + 1) * ITILE)
                s_ps = psS.tile([P, 2, ITILE], F32, tag="ps")
                for par in range(2):
                    jc = 2 * pr + par
                    jsl = slice(jc * P, (jc + 1) * P)
                    nc.tensor.matmul(s_ps[:, par, :], lhsT=k_sb[:, :, jsl],
                                     rhs=q_sb[:, :, isl],
                                     perf_mode=DR, start=True, stop=True)
                p2 = pp.tile([P, 2, ITILE], FP8, tag="p")
                nc.scalar.activation(out=p2, in_=s_ps, func=AF.Exp, scale=SCALE)
                return p2

            def emit_pv(pr, p2, o_ps0, o_ps1, l_ps):
                st, sp = (pr == 0), (pr == NPR - 1)
                nc.tensor.matmul(o_ps0, lhsT=v_sb[:, 2 * pr:2 * pr + 2, 0:P],
                                 rhs=p2, perf_mode=DR, start=st, stop=sp)
                nc.tensor.matmul(o_ps1, lhsT=v_sb[:, 2 * pr:2 * pr + 2, P:C],
                                 rhs=p2, perf_mode=DR, start=st, stop=sp)
                nc.tensor.matmul(l_ps, lhsT=ones8, rhs=p2,
                                 perf_mode=DR, start=st, stop=sp)

            def make_tail_a(it, o_ps0, o_ps1, l_ps):
                def tail_a():
                    recip = rp.tile([P, ITILE], F32, tag="recip")
                    nc.vector.reciprocal_approx_fast(out=recip, in_=l_ps)
                    o_sb = op_pool.tile([P, 2, ITILE], BF16, tag="osb")
                    nc.vector.tensor_tensor(out=o_sb[:, 0], in0=o_ps0,
                                            in1=recip, op=OP.mult)
                    nc.vector.tensor_tensor(out=o_sb[:, 1], in0=o_ps1,
                                            in1=recip, op=OP.mult)
                    return o_sb
                return tail_a

            def make_tail_b(it, o_sb):
                def tail_b():
                    isl = slice(it * ITILE, (it + 1) * ITILE)
                    # output projection + bias + residual
                    for oo in range(2):
                        u_ps = ps1()
                        for ci in range(2):
                            nc.tensor.matmul(
                                u_ps, lhsT=wo_sb[:, ci, oo * P:(oo + 1) * P],
                                rhs=o_sb[:, ci],
                                start=(ci == 0), stop=(ci == 1))
                        res = resp.tile([P, ITILE], F32, tag="res")
                        nc.vector.scalar_tensor_tensor(
                            out=res, in0=u_ps, scalar=bo_sb[:, oo:oo + 1],
                            in1=xres_sb[:, oo, isl], op0=OP.add, op1=OP.add)
                        nc.sync.dma_start(
                            out=out_d[:].rearrange("(o p) i -> p o i", p=P)[:, oo, isl],
                            in_=res)
                return tail_b

            pending_a = pending_b = None
            for it in range(NIT):
                o_ps0 = psO.tile([P, ITILE], F32, tag="o0")
                o_ps1 = psO.tile([P, ITILE], F32, tag="o1")
                l_ps = psL.tile([P, ITILE], F32, tag="lps")
                pq = []
                for pr in range(NPR):
                    pq.append(emit_s_exp(it, pr))
                    if pr >= 2:
                        emit_pv(pr - 2, pq[pr - 2], o_ps0, o_ps1, l_ps)
                    if pending_a is not None and pr == 2:
                        o_sb_prev = pending_a()
                        pending_b = make_tail_b(it - 1, o_sb_prev)
                        pending_a = None
                    if pending_b is not None and pr == 5:
                        pending_b()
                        pending_b = None
                emit_pv(NPR - 2, pq[NPR - 2], o_ps0, o_ps1, l_ps)
                emit_pv(NPR - 1, pq[NPR - 1], o_ps0, o_ps1, l_ps)
                pending_a = make_tail_a(it, o_ps0, o_ps1, l_ps)
            o_sb_last = pending_a()
            make_tail_b(NIT - 1, o_sb_last)()

    nc.compile()
    return nc


def _get_prog():
    global _PROG
    if _PROG is None:
        _PROG = _build()
    return _PROG


def kernel(x, gn_w, gn_b, wq, bq, wk, bk, wv, bv, wo, bo):
    global _LAST_RESULTS
    import ml_dtypes
    from concourse.bass_utils import run_bass_kernel_spmd

    nc = _get_prog()

    f32 = lambda a: np.ascontiguousarray(np.asarray(a), dtype=np.float32)
    bf16 = lambda a: np.ascontiguousarray(np.asarray(a, dtype=np.float32).astype(ml_dtypes.bfloat16))
    x = f32(x).reshape(B, C, N)
    shared = {
        "wqT": bf16(np.asarray(wq).T), "wkT": bf16(np.asarray(wk).T),
        "wvT": bf16(np.asarray(wv).T), "woT": bf16(np.asarray(wo).T),
        "bq": f32(bq), "bk": f32(bk), "bv": f32(bv), "bo": f32(bo),
        "gnw": f32(gn_w), "gnb": f32(gn_b),
    }
    a8 = np.zeros((C, GROUPS), np.float32)
    a8[np.arange(C), np.arange(C) // GSIZE] = 1.0 / GSIZE
    shared["a8"] = a8
    e8 = np.zeros((P, C), np.float32)
    e8[np.arange(C) // GSIZE, np.arange(C)] = 1.0
    shared["e8"] = e8

    in_maps = []
    for core in range(NCORES):
        b, h = core // 2, core % 2
        xb = x[b]
        if h == 0:
            xc = xb
        else:
            xc = np.ascontiguousarray(np.concatenate([xb[:, HALF:], xb[:, :HALF]], axis=1))
        m = dict(shared)
        m["xbf"] = bf16(xc)
        m["xres"] = np.ascontiguousarray(xb[:, h * HALF:(h + 1) * HALF])
        in_maps.append(m)

    _LAST_RESULTS = run_bass_kernel_spmd(nc, in_maps, list(range(NCORES)),
                                         trace=_TRACE)
    out = np.empty((B, C, N), np.float32)
    for core in range(NCORES):
        b, h = core // 2, core % 2
        out[b, :, h * HALF:(h + 1) * HALF] = _LAST_RESULTS.results[core]["out"]
    return out.reshape(B, C, 16, 16, 16)
